# revision 29
# baseline (speedup 1.0000x reference)
"""GAT encoder layer (nn_GATencoderlayer) on 8 Trainium2 NeuronCores.

Sharding: data-parallel over batch (B=8 -> 1 batch element per core),
params replicated. No collectives needed.

Math notes (per batch element, N=2048 nodes, D=256):
  h      = feat @ W0                       [N, D]
  a_src  = feat @ (W0 @ ws_src)  (reassociated, param-only host prefold)
  a_dst  = feat @ (W0 @ ws_dst) + bs
  x[i,j] = a_src[i] + a_dst[j] + bs
  attn   = sigmoid(x) on strict lower triangle (j < i)
  I_A    = I - attn                 (second output, diag == 1, upper == 0)
  scores = where(tril & adj, I_A, NEG);  P = softmax(scores, axis=-1)
  feat_out = P @ h + b; elu; gate = sigmoid(feat @ Hw.T + Hb); blend.

Device computes everything in the transposed (j=source-major) orientation to
avoid on-chip transposes:
  sigmoid(x) = 0.5 + 0.5*tanh(x/2)  (tanh+exp share one ACT table set)
  e[j,i] = c*exp(-sigmoid(x)) * adjT   with c = exp(-0.5):
         = exp(-0.5*tanh(x/2)) * M1[j,i],  M1 = c*adjT*strict_mask (host)
  diag:  e[j,j] = c*e^1*adj[j,j]  added via M2 (host-built diagonal tiles)
  P@h is computed unnormalized (numer, den) plus a uniform delta offset
  (delta=1e-11) that vanishes in fp32 adds for any nonzero e but reproduces
  the reference softmax's uniform-distribution behavior for fully-masked
  rows exactly: (numer + delta*colsum(h_aug)) / (den + delta*N).
  I_A^T[j,i] = -0.5 - 0.5*tanh(x/2) for j<i; written transposed, the host
  re-views it as [i,j] and applies the structural tril/diag mask.
"""

import math
from contextlib import ExitStack

import numpy as np

B, N, D = 8, 2048, 256
P = 128
NT = N // P  # 16 tiles of 128
NCORES = 8
DELTA = 1e-11
C_SCALE = math.exp(-0.5)
E_DIAG = math.exp(1.0)  # exp(I_A[i,i]) = exp(1); M1's C_SCALE factor merely
# reconstructs exp(-sigma) = C_SCALE * exp(-0.5*tanh(x/2)), it is not a
# global softmax rescale, so the diagonal term carries no C_SCALE.

_PROGRAM_CACHE = {}


def _build_program():
    import concourse.bacc as bacc
    import concourse.tile as tile
    from concourse import mybir

    f32 = mybir.dt.float32
    AF = mybir.ActivationFunctionType
    OP = mybir.AluOpType

    nc = bacc.Bacc()

    # ---- per-core I/O ----------------------------------------------------
    bf16 = mybir.dt.bfloat16
    featT = nc.dram_tensor("featT", [D, N], f32, kind="ExternalInput")
    feat = nc.dram_tensor("feat", [N, D], f32, kind="ExternalInput")
    M1 = nc.dram_tensor("m1", [N, N], bf16, kind="ExternalInput")
    M2 = nc.dram_tensor("m2", [N, P], bf16, kind="ExternalInput")
    Wbig = nc.dram_tensor("wbig", [D, D + 3], f32, kind="ExternalInput")
    HwT = nc.dram_tensor("hwt", [D, D], bf16, kind="ExternalInput")
    HbR = nc.dram_tensor("hbr", [1, D], bf16, kind="ExternalInput")
    BbT = nc.dram_tensor("bb", [P, D], f32, kind="ExternalInput")
    Crow = nc.dram_tensor("crow", [P, D + 3], f32, kind="ExternalInput")
    BsCol = nc.dram_tensor("bscol", [P, 1], f32, kind="ExternalInput")
    IAT = nc.dram_tensor("iat", [N, N], f32, kind="ExternalOutput")
    outb = nc.dram_tensor("outb", [N, D], f32, kind="ExternalOutput")

    DP3 = D + 3  # 259: [h(256) | a_src | a_dst+bs | ones]

    with tile.TileContext(nc) as tc, ExitStack() as ctx:
        psum = ctx.enter_context(tc.tile_pool(name="psum", bufs=8, space="PSUM"))
        singles = ctx.enter_context(tc.tile_pool(name="singles", bufs=1))
        wpool = ctx.enter_context(tc.tile_pool(name="wpool", bufs=2))
        iapool = ctx.enter_context(tc.tile_pool(name="iapool", bufs=2))
        m1pool = ctx.enter_context(tc.tile_pool(name="m1pool", bufs=3))
        fpool = ctx.enter_context(tc.tile_pool(name="fpool", bufs=2))
        tpool = ctx.enter_context(tc.tile_pool(name="tpool", bufs=2))

        # ---- constants / params -----------------------------------------
        ft0 = singles.tile([P, N], f32, tag="ft0")
        ft1 = singles.tile([P, N], f32, tag="ft1")
        nc.sync.dma_start(out=ft0, in_=featT[0:P, :])
        nc.sync.dma_start(out=ft1, in_=featT[P : 2 * P, :])

        wb0 = singles.tile([P, DP3], f32, tag="wb0")
        wb1 = singles.tile([P, DP3], f32, tag="wb1")
        nc.sync.dma_start(out=wb0, in_=Wbig[0:P, :])
        nc.sync.dma_start(out=wb1, in_=Wbig[P : 2 * P, :])

        hwt0 = singles.tile([P, D], bf16, tag="hwt0")
        hwt1 = singles.tile([P, D], bf16, tag="hwt1")
        nc.sync.dma_start(out=hwt0, in_=HwT[0:P, :])
        nc.sync.dma_start(out=hwt1, in_=HwT[P : 2 * P, :])

        hb_row = singles.tile([1, D], bf16, tag="hb_row")
        nc.sync.dma_start(out=hb_row, in_=HbR[:, :])
        bb_sb = singles.tile([P, D], f32, tag="bb_sb")
        nc.sync.dma_start(out=bb_sb, in_=BbT[:, :])
        bs_col = singles.tile([P, 1], f32, tag="bs_col")
        nc.sync.dma_start(out=bs_col, in_=BsCol[:, :])


        m2_sb = singles.tile([P, NT, P], bf16, tag="m2_sb")
        nc.sync.dma_start(out=m2_sb, in_=M2.rearrange("(t p) q -> p t q", p=P))
        feat_sb = singles.tile([P, NT, D], f32, tag="feat_sb")
        nc.sync.dma_start(out=feat_sb, in_=feat.rearrange("(t p) c -> p t c", p=P))

        ones_row = singles.tile([1, P], f32, tag="ones_row")
        nc.vector.memset(ones_row, 1.0)
        ones_row16 = singles.tile([1, P], bf16, tag="ones_row16")
        nc.vector.memset(ones_row16, 1.0)
        ones_col16 = singles.tile([P, 1], bf16, tag="ones_col16")
        nc.vector.memset(ones_col16, 1.0)
        neg_half = singles.tile([P, 1], f32, tag="neg_half")
        nc.vector.memset(neg_half, -0.5)

        # bf16 copies of featT for the gate matmuls
        ft16_0 = singles.tile([P, N], bf16, tag="ft16_0")
        ft16_1 = singles.tile([P, N], bf16, tag="ft16_1")
        nc.vector.tensor_copy(out=ft16_0, in_=ft0)
        nc.vector.tensor_copy(out=ft16_1, in_=ft1)

        # ---- a_src as a row + broadcast across partitions ---------------
        asr_sb = singles.tile([1, N], f32, tag="asr_sb")
        for blk in range(4):
            ps = psum.tile([P, 512], f32, tag="acc")
            sl = slice(512 * blk, 512 * blk + 512)
            nc.tensor.matmul(ps[0:1, 0:512], lhsT=wb0[:, D : D + 1], rhs=ft0[:, sl], start=True, stop=False)
            nc.tensor.matmul(ps[0:1, 0:512], lhsT=wb1[:, D : D + 1], rhs=ft1[:, sl], start=False, stop=True)
            nc.scalar.copy(out=asr_sb[0:1, sl], in_=ps[0:1, 0:512])

        a_src_b = singles.tile([P, N], f32, tag="a_src_b")
        for blk in range(4):
            ps = psum.tile([P, 512], f32, tag="acc")
            sl = slice(512 * blk, 512 * blk + 512)
            nc.tensor.matmul(ps[:, 0:512], lhsT=ones_row, rhs=asr_sb[0:1, sl], start=True, stop=True)
            nc.scalar.copy(out=a_src_b[:, sl], in_=ps[:, 0:512])

        # ---- h16[j, 0:256]=h, 258=1 (bf16 rhs for P@h); bias_half fp32 ----
        # bias_half[j] = 0.5*(a_dst[j] + bs), fp32-exact for the I_A path
        h16 = singles.tile([P, NT, DP3], bf16, tag="h16")
        nc.vector.memset(h16[:, :, D : D + 3], 1.0)
        bias_half = singles.tile([P, NT], f32, tag="bias_half")
        for jt in range(NT):
            ps = psum.tile([P, 512], f32, tag="acc")
            sl = slice(P * jt, P * jt + P)
            nc.tensor.matmul(ps[:, 0:DP3], lhsT=ft0[:, sl], rhs=wb0, start=True, stop=False)
            nc.tensor.matmul(ps[:, 0:DP3], lhsT=ft1[:, sl], rhs=wb1, start=False, stop=True)
            nc.scalar.copy(out=h16[:, jt, 0:D], in_=ps[:, 0:D])
            nc.vector.tensor_scalar(
                out=bias_half[:, jt : jt + 1], in0=ps[:, D + 1 : D + 2],
                scalar1=bs_col, scalar2=0.5, op0=OP.add, op1=OP.mult,
            )

        # ---- column sums of h_aug -> extra_row = delta * S --------------
        sp = psum.tile([P, 512], f32, tag="acc")
        for jt in range(NT):
            nc.tensor.matmul(
                sp[0:1, 0:DP3], lhsT=ones_col16, rhs=h16[:, jt, :],
                start=(jt == 0), stop=(jt == NT - 1),
            )
        extra_row = singles.tile([1, DP3], bf16, tag="extra_row")
        nc.vector.tensor_scalar_mul(out=extra_row, in0=sp[0:1, 0:DP3], scalar1=DELTA)

        # ---- gate matmuls early: dense PE block, results parked in tg_all
        tg_all = singles.tile([P, NT, D], f32, tag="tg_all")
        for pair in range(NT // 2):
            gps = psum.tile([P, 512], f32, tag="acc", name=f"gps{pair}")
            for k in range(2):
                it = 2 * pair + k
                osl = slice(D * k, D * k + D)
                isl = slice(P * it, P * it + P)
                nc.tensor.matmul(gps[:, osl], lhsT=ft16_0[:, isl], rhs=hwt0, start=True, stop=False)
                nc.tensor.matmul(gps[:, osl], lhsT=ft16_1[:, isl], rhs=hwt1, start=False, stop=False)
                nc.tensor.matmul(gps[:, osl], lhsT=ones_row16, rhs=hb_row, start=False, stop=True)
            nc.scalar.activation(
                out=tg_all[:, 2 * pair : 2 * pair + 2, :], in_=gps[:, 0:512],
                func=AF.Tanh, scale=0.5,
            )

        # ---- row production: full-width rows, retained in SBUF ----------
        fpre_tiles = {}
        erows = {}

        def emit_tail(g):
            # tail over row-tiles [i0, i0+cnt)
            i0, cnt = tail_groups[g]
            fpre_g = fpre_tiles[g]
            fsl = feat_sb[:, i0 : i0 + cnt, :]
            n1 = tpool.tile([P, 2, D], f32, tag="n1", bufs=2, name="n1")[:, 0:cnt, :]
            # n1 = max(-x, 0)  (DVE 2x tensor_scalar)
            nc.vector.tensor_scalar(
                out=n1, in0=fpre_g, scalar1=-1.0, scalar2=0.0, op0=OP.mult, op1=OP.max
            )
            eex = tpool.tile([P, 2, D], f32, tag="eex", bufs=2, name="eex")[:, 0:cnt, :]
            nc.scalar.activation(out=eex, in_=n1, func=AF.Exp, scale=-1.0)
            # eex <- d = eex - 1 + n1  (= elu(x) - x)
            nc.vector.scalar_tensor_tensor(
                out=eex, in0=eex, scalar=-1.0, in1=n1, op0=OP.add, op1=OP.add
            )
            xmf = tpool.tile([P, 2, D], f32, tag="xmf", bufs=2, name="xmf")[:, 0:cnt, :]
            nc.gpsimd.tensor_sub(out=xmf, in0=fpre_g, in1=fsl)
            # xmf <- emf = elu(x) - feat_in
            nc.gpsimd.tensor_add(out=xmf, in0=eex, in1=xmf)
            # xmf <- (tanh + 1) * (elu - feat)   [gate = 0.5*(tanh+1)]
            nc.vector.scalar_tensor_tensor(
                out=xmf, in0=tg_all[:, i0 : i0 + cnt, :], scalar=1.0,
                in1=xmf, op0=OP.add, op1=OP.mult,
            )
            # xmf <- feat + 0.5 * (tanh+1)*(elu-feat)
            nc.vector.scalar_tensor_tensor(
                out=xmf, in0=xmf, scalar=0.5, in1=fsl, op0=OP.mult, op1=OP.add
            )
            nc.sync.dma_start(
                out=outb.rearrange("(t p) c -> p t c", p=P)[:, i0 : i0 + cnt, :],
                in_=xmf,
            )

        for jt in range(NT):
            j0 = P * jt
            W = N - j0
            w_t = wpool.tile([P, N], f32, tag="w")
            nc.scalar.activation(
                out=w_t[:, :W], in_=a_src_b[:, j0:N], func=AF.Tanh,
                bias=bias_half[:, jt : jt + 1], scale=0.5,
            )
            ia_t = iapool.tile([P, N], f32, tag="ia")
            nc.vector.tensor_scalar(
                out=ia_t[:, :W], in0=w_t[:, :W], scalar1=-0.5, scalar2=-0.5,
                op0=OP.mult, op1=OP.add,
            )
            nc.sync.dma_start(out=IAT[j0 : j0 + P, j0:N], in_=ia_t[:, :W])

            e_t = singles.tile([P, W], bf16, tag=f"erow{jt}", name=f"erow{jt}")
            nc.scalar.activation(
                out=e_t, in_=w_t[:, :W], func=AF.Exp, scale=-0.5, bias=neg_half
            )
            m1_t = m1pool.tile([P, N], bf16, tag="m1")
            nc.sync.dma_start(out=m1_t[:, :W], in_=M1[j0 : j0 + P, j0:N])
            nc.vector.tensor_mul(out=e_t, in0=e_t, in1=m1_t[:, :W])
            nc.vector.tensor_add(out=e_t[:, 0:P], in0=e_t[:, 0:P], in1=m2_sb[:, jt, :])
            erows[jt] = e_t

        # tails: pairs for early tiles, singles for the last four (shorter
        # critical chains where they land on the kernel's tail)
        tail_groups = [(0, 2), (2, 2), (4, 2), (6, 2), (8, 2), (10, 2),
                       (12, 1), (13, 1), (14, 1), (15, 1)]
        it_to_group = {}
        for gi, (i0, cnt) in enumerate(tail_groups):
            for k in range(cnt):
                it_to_group[i0 + k] = gi

        # ---- P@h accumulation: dense matmul stream over retained rows ----
        for it in range(NT):
            acc = psum.tile([P, 512], f32, tag="acc", name=f"acc{it}")
            for jt in range(it + 1):
                off = P * (it - jt)
                nc.tensor.matmul(
                    acc[:, 0:DP3], lhsT=erows[jt][:, off : off + P],
                    rhs=h16[:, jt, :], start=(jt == 0), stop=False,
                )
            nc.tensor.matmul(
                acc[:, 0:DP3], lhsT=ones_row16, rhs=extra_row,
                start=False, stop=True,
            )
            g = it_to_group[it]
            i0, cnt = tail_groups[g]
            if it == i0:
                fpre_tiles[g] = fpool.tile(
                    [P, 2, D], f32, tag="fpre", name=f"fpre{g}",
                )[:, 0:cnt, :]
            rec = tpool.tile([P, 1], f32, tag="rec")
            nc.vector.reciprocal(out=rec, in_=acc[:, DP3 - 1 : DP3])
            nc.vector.scalar_tensor_tensor(
                out=fpre_tiles[g][:, it - i0, :], in0=acc[:, 0:D], scalar=rec,
                in1=bb_sb, op0=OP.mult, op1=OP.add,
            )
            if it == i0 + cnt - 1:
                emit_tail(g)

    nc.finalize()
    return nc


def _get_program():
    if "nc" not in _PROGRAM_CACHE:
        _PROGRAM_CACHE["nc"] = _build_program()
    return _PROGRAM_CACHE["nc"]


def _host_prep(feat_in, adj, W, b, Ws, bs, Hw, Hb):
    """Build the per-core input maps (layout marshalling + param prefolds)."""
    f4 = np.float32
    W0 = np.asarray(W, f4)[0]  # [256, 256] (in, out)
    ws_src = np.asarray(Ws, f4)[0, :D]
    ws_dst = np.asarray(Ws, f4)[0, D:]
    u_src = W0 @ ws_src
    u_dst = W0 @ ws_dst
    wbig = np.concatenate(
        [W0, u_src[:, None], u_dst[:, None], np.zeros((D, 1), f4)], axis=1
    ).astype(f4)
    crow = np.zeros((1, D + 3), f4)
    crow[0, D + 2] = 1.0  # ones column for h_aug (bs handled separately)
    hwt = np.ascontiguousarray(np.asarray(Hw, f4).T)
    hbr = np.asarray(Hb, f4).reshape(1, D)
    bb = np.ascontiguousarray(np.broadcast_to(np.asarray(b, f4), (P, D)))

    import ml_dtypes

    bf16 = ml_dtypes.bfloat16
    strict = np.triu(np.ones((N, N), dtype=f4), 1)  # [j, i] : j < i
    in_maps = []
    for bi in range(B):
        fb = np.ascontiguousarray(np.asarray(feat_in[bi], f4))
        fbT = np.ascontiguousarray(fb.T)
        adjb = np.asarray(adj[bi])
        # 0/1 keep-mask (exact in bf16); exp(-0.5) lives in the ACT exp bias
        m1 = np.ascontiguousarray((adjb.T.astype(f4)) * strict).astype(bf16)
        dvals = (np.diagonal(adjb).astype(f4)) * f4(E_DIAG)
        m2 = np.zeros((N, P), f4)
        m2[np.arange(N), np.arange(N) % P] = dvals
        m2 = m2.astype(bf16)
        in_maps.append(
            {
                "featT": fbT,
                "feat": fb,
                "m1": m1,
                "m2": m2,
                "wbig": wbig,
                "hwt": hwt.astype(bf16),
                "hbr": hbr.astype(bf16),
                "bb": bb,
                "crow": np.ascontiguousarray(np.broadcast_to(crow, (P, D + 3))),
                "bscol": np.full((P, 1), np.asarray(bs, f4)[0], f4),
            }
        )
    return in_maps


def _assemble(results):
    """Gather per-core outputs into full tensors (layout-only host work)."""
    f4 = np.float32
    out = np.stack([results[i]["outb"] for i in range(B)], axis=0)
    iat = np.stack([results[i]["iat"] for i in range(B)], axis=0)  # [B, j, i]
    ia_ij = iat.transpose(0, 2, 1)  # [B, i, j] view
    strict_low = np.tril(np.ones((N, N), dtype=bool), -1)
    eye = np.eye(N, dtype=f4)
    i_a_raw = np.where(strict_low[None], ia_ij, eye[None]).astype(f4)[:, None]
    return out.astype(f4), i_a_raw


def run_on_cores(in_maps, trace=False, **kwargs):
    from concourse.bass_utils import run_bass_kernel_spmd

    nc = _get_program()
    return run_bass_kernel_spmd(nc, in_maps, list(range(NCORES)), trace=trace, **kwargs)


def kernel(feat_in, adj, W, b, Ws, bs, Hw, Hb):
    in_maps = _host_prep(feat_in, adj, W, b, Ws, bs, Hw, Hb)
    res = run_on_cores(in_maps, trace=False)
    return _assemble(res.results)


# revision 30
# speedup vs baseline: 1.1053x; 1.1053x over previous
"""GAT encoder layer (nn_GATencoderlayer) on 8 Trainium2 NeuronCores.

Sharding: data-parallel over batch (B=8 -> 1 batch element per core),
params replicated. No collectives needed.

Math notes (per batch element, N=2048 nodes, D=256):
  h      = feat @ W0                       [N, D]
  a_src  = feat @ (W0 @ ws_src)  (reassociated, param-only host prefold)
  a_dst  = feat @ (W0 @ ws_dst) + bs
  x[i,j] = a_src[i] + a_dst[j] + bs
  attn   = sigmoid(x) on strict lower triangle (j < i)
  I_A    = I - attn                 (second output, diag == 1, upper == 0)
  scores = where(tril & adj, I_A, NEG);  P = softmax(scores, axis=-1)
  feat_out = P @ h + b; elu; gate = sigmoid(feat @ Hw.T + Hb); blend.

Device computes everything in the transposed (j=source-major) orientation to
avoid on-chip transposes:
  sigmoid(x) = 0.5 + 0.5*tanh(x/2)  (tanh+exp share one ACT table set)
  e[j,i] = c*exp(-sigmoid(x)) * adjT   with c = exp(-0.5):
         = exp(-0.5*tanh(x/2)) * M1[j,i],  M1 = c*adjT*strict_mask (host)
  diag:  e[j,j] = c*e^1*adj[j,j]  added via M2 (host-built diagonal tiles)
  P@h is computed unnormalized (numer, den) plus a uniform delta offset
  (delta=1e-11) that vanishes in fp32 adds for any nonzero e but reproduces
  the reference softmax's uniform-distribution behavior for fully-masked
  rows exactly: (numer + delta*colsum(h_aug)) / (den + delta*N).
  I_A^T[j,i] = -0.5 - 0.5*tanh(x/2) for j<i; written transposed, the host
  re-views it as [i,j] and applies the structural tril/diag mask.
"""

import math
from contextlib import ExitStack

import numpy as np

B, N, D = 8, 2048, 256
P = 128
NT = N // P  # 16 tiles of 128
NCORES = 8
DELTA = 1e-11
C_SCALE = math.exp(-0.5)
E_DIAG = math.exp(1.0)  # exp(I_A[i,i]) = exp(1); M1's C_SCALE factor merely
# reconstructs exp(-sigma) = C_SCALE * exp(-0.5*tanh(x/2)), it is not a
# global softmax rescale, so the diagonal term carries no C_SCALE.

_PROGRAM_CACHE = {}


def _build_program():
    import concourse.bacc as bacc
    import concourse.tile as tile
    from concourse import mybir

    f32 = mybir.dt.float32
    AF = mybir.ActivationFunctionType
    OP = mybir.AluOpType

    nc = bacc.Bacc()

    # ---- per-core I/O ----------------------------------------------------
    bf16 = mybir.dt.bfloat16
    featT = nc.dram_tensor("featT", [D, N], f32, kind="ExternalInput")
    feat = nc.dram_tensor("feat", [N, D], f32, kind="ExternalInput")
    M1 = nc.dram_tensor("m1", [N, N], bf16, kind="ExternalInput")
    M2 = nc.dram_tensor("m2", [N, P], bf16, kind="ExternalInput")
    Wbig = nc.dram_tensor("wbig", [D, D + 3], f32, kind="ExternalInput")
    HwT = nc.dram_tensor("hwt", [D, D], bf16, kind="ExternalInput")
    HbR = nc.dram_tensor("hbr", [1, D], bf16, kind="ExternalInput")
    BbT = nc.dram_tensor("bb", [P, D], f32, kind="ExternalInput")
    Crow = nc.dram_tensor("crow", [P, D + 3], f32, kind="ExternalInput")
    BsCol = nc.dram_tensor("bscol", [P, 1], f32, kind="ExternalInput")
    IAT = nc.dram_tensor("iat", [N, N], f32, kind="ExternalOutput")
    outb = nc.dram_tensor("outb", [N, D], f32, kind="ExternalOutput")

    DP3 = D + 3  # 259: [h(256) | a_src | a_dst+bs | ones]

    with tile.TileContext(nc) as tc, ExitStack() as ctx:
        psum = ctx.enter_context(tc.tile_pool(name="psum", bufs=8, space="PSUM"))
        singles = ctx.enter_context(tc.tile_pool(name="singles", bufs=1))
        wpool = ctx.enter_context(tc.tile_pool(name="wpool", bufs=3))
        iapool = ctx.enter_context(tc.tile_pool(name="iapool", bufs=3))
        m1pool = ctx.enter_context(tc.tile_pool(name="m1pool", bufs=3))
        fpool = ctx.enter_context(tc.tile_pool(name="fpool", bufs=2))
        tpool = ctx.enter_context(tc.tile_pool(name="tpool", bufs=2))

        # ---- constants / params -----------------------------------------
        ft0 = singles.tile([P, N], f32, tag="ft0")
        ft1 = singles.tile([P, N], f32, tag="ft1")
        nc.sync.dma_start(out=ft0, in_=featT[0:P, :])
        nc.sync.dma_start(out=ft1, in_=featT[P : 2 * P, :])

        wb0 = singles.tile([P, DP3], f32, tag="wb0")
        wb1 = singles.tile([P, DP3], f32, tag="wb1")
        nc.sync.dma_start(out=wb0, in_=Wbig[0:P, :])
        nc.sync.dma_start(out=wb1, in_=Wbig[P : 2 * P, :])

        hwt0 = singles.tile([P, D], bf16, tag="hwt0")
        hwt1 = singles.tile([P, D], bf16, tag="hwt1")
        nc.sync.dma_start(out=hwt0, in_=HwT[0:P, :])
        nc.sync.dma_start(out=hwt1, in_=HwT[P : 2 * P, :])

        hb_row = singles.tile([1, D], bf16, tag="hb_row")
        nc.sync.dma_start(out=hb_row, in_=HbR[:, :])
        bb_sb = singles.tile([P, D], f32, tag="bb_sb")
        nc.sync.dma_start(out=bb_sb, in_=BbT[:, :])
        bs_col = singles.tile([P, 1], f32, tag="bs_col")
        nc.sync.dma_start(out=bs_col, in_=BsCol[:, :])


        m2_sb = singles.tile([P, NT, P], bf16, tag="m2_sb")
        nc.sync.dma_start(out=m2_sb, in_=M2.rearrange("(t p) q -> p t q", p=P))
        feat_sb = singles.tile([P, NT, D], f32, tag="feat_sb")
        nc.sync.dma_start(out=feat_sb, in_=feat.rearrange("(t p) c -> p t c", p=P))

        ones_row = singles.tile([1, P], f32, tag="ones_row")
        nc.vector.memset(ones_row, 1.0)
        ones_row16 = singles.tile([1, P], bf16, tag="ones_row16")
        nc.vector.memset(ones_row16, 1.0)
        ones_col16 = singles.tile([P, 1], bf16, tag="ones_col16")
        nc.vector.memset(ones_col16, 1.0)
        neg_half = singles.tile([P, 1], f32, tag="neg_half")
        nc.vector.memset(neg_half, -0.5)

        # bf16 copies of featT for the gate matmuls
        ft16_0 = singles.tile([P, N], bf16, tag="ft16_0")
        ft16_1 = singles.tile([P, N], bf16, tag="ft16_1")
        nc.vector.tensor_copy(out=ft16_0, in_=ft0)
        nc.vector.tensor_copy(out=ft16_1, in_=ft1)

        # ---- a_src as a row + broadcast across partitions ---------------
        asr_sb = singles.tile([1, N], f32, tag="asr_sb")
        for blk in range(4):
            ps = psum.tile([P, 512], f32, tag="acc")
            sl = slice(512 * blk, 512 * blk + 512)
            nc.tensor.matmul(ps[0:1, 0:512], lhsT=wb0[:, D : D + 1], rhs=ft0[:, sl], start=True, stop=False)
            nc.tensor.matmul(ps[0:1, 0:512], lhsT=wb1[:, D : D + 1], rhs=ft1[:, sl], start=False, stop=True)
            nc.scalar.copy(out=asr_sb[0:1, sl], in_=ps[0:1, 0:512])

        a_src_b = singles.tile([P, N], f32, tag="a_src_b")
        for blk in range(4):
            ps = psum.tile([P, 512], f32, tag="acc")
            sl = slice(512 * blk, 512 * blk + 512)
            nc.tensor.matmul(ps[:, 0:512], lhsT=ones_row, rhs=asr_sb[0:1, sl], start=True, stop=True)
            nc.scalar.copy(out=a_src_b[:, sl], in_=ps[:, 0:512])

        # ---- h16[j, 0:256]=h, 258=1 (bf16 rhs for P@h); bias_half fp32 ----
        # bias_half[j] = 0.5*(a_dst[j] + bs), fp32-exact for the I_A path
        h16 = singles.tile([P, NT, DP3], bf16, tag="h16")
        nc.vector.memset(h16[:, :, D : D + 3], 1.0)
        bias_half = singles.tile([P, NT], f32, tag="bias_half")
        for jt in range(NT):
            ps = psum.tile([P, 512], f32, tag="acc")
            sl = slice(P * jt, P * jt + P)
            nc.tensor.matmul(ps[:, 0:DP3], lhsT=ft0[:, sl], rhs=wb0, start=True, stop=False)
            nc.tensor.matmul(ps[:, 0:DP3], lhsT=ft1[:, sl], rhs=wb1, start=False, stop=True)
            nc.scalar.copy(out=h16[:, jt, 0:D], in_=ps[:, 0:D])
            nc.vector.tensor_scalar(
                out=bias_half[:, jt : jt + 1], in0=ps[:, D + 1 : D + 2],
                scalar1=bs_col, scalar2=0.5, op0=OP.add, op1=OP.mult,
            )

        # ---- column sums of h_aug -> extra_row = delta * S --------------
        sp = psum.tile([P, 512], f32, tag="acc")
        for jt in range(NT):
            nc.tensor.matmul(
                sp[0:1, 0:DP3], lhsT=ones_col16, rhs=h16[:, jt, :],
                start=(jt == 0), stop=(jt == NT - 1),
            )
        extra_row = singles.tile([1, DP3], bf16, tag="extra_row")
        nc.vector.tensor_scalar_mul(out=extra_row, in0=sp[0:1, 0:DP3], scalar1=DELTA)

        # ---- gate matmuls early: dense PE block, results parked in tg_all
        tg_all = singles.tile([P, NT, D], f32, tag="tg_all")
        for pair in range(NT // 2):
            gps = psum.tile([P, 512], f32, tag="acc", name=f"gps{pair}")
            for k in range(2):
                it = 2 * pair + k
                osl = slice(D * k, D * k + D)
                isl = slice(P * it, P * it + P)
                nc.tensor.matmul(gps[:, osl], lhsT=ft16_0[:, isl], rhs=hwt0, start=True, stop=False)
                nc.tensor.matmul(gps[:, osl], lhsT=ft16_1[:, isl], rhs=hwt1, start=False, stop=False)
                nc.tensor.matmul(gps[:, osl], lhsT=ones_row16, rhs=hb_row, start=False, stop=True)
            nc.scalar.activation(
                out=tg_all[:, 2 * pair : 2 * pair + 2, :], in_=gps[:, 0:512],
                func=AF.Tanh, scale=0.5,
            )

        # ---- row production: full-width rows, retained in SBUF ----------
        fpre_tiles = {}
        erows = {}

        def emit_tail(g):
            # tail over row-tiles [i0, i0+cnt)
            i0, cnt = tail_groups[g]
            fpre_g = fpre_tiles[g]
            fsl = feat_sb[:, i0 : i0 + cnt, :]
            n1 = tpool.tile([P, 2, D], f32, tag="n1", bufs=2, name="n1")[:, 0:cnt, :]
            # n1 = max(-x, 0)  (DVE 2x tensor_scalar)
            nc.vector.tensor_scalar(
                out=n1, in0=fpre_g, scalar1=-1.0, scalar2=0.0, op0=OP.mult, op1=OP.max
            )
            eex = tpool.tile([P, 2, D], f32, tag="eex", bufs=2, name="eex")[:, 0:cnt, :]
            nc.scalar.activation(out=eex, in_=n1, func=AF.Exp, scale=-1.0)
            # eex <- d = eex - 1 + n1  (= elu(x) - x)
            nc.vector.scalar_tensor_tensor(
                out=eex, in0=eex, scalar=-1.0, in1=n1, op0=OP.add, op1=OP.add
            )
            xmf = tpool.tile([P, 2, D], f32, tag="xmf", bufs=2, name="xmf")[:, 0:cnt, :]
            nc.gpsimd.tensor_sub(out=xmf, in0=fpre_g, in1=fsl)
            # xmf <- emf = elu(x) - feat_in
            nc.gpsimd.tensor_add(out=xmf, in0=eex, in1=xmf)
            # xmf <- (tanh + 1) * (elu - feat)   [gate = 0.5*(tanh+1)]
            nc.vector.scalar_tensor_tensor(
                out=xmf, in0=tg_all[:, i0 : i0 + cnt, :], scalar=1.0,
                in1=xmf, op0=OP.add, op1=OP.mult,
            )
            # xmf <- feat + 0.5 * (tanh+1)*(elu-feat)
            nc.vector.scalar_tensor_tensor(
                out=xmf, in0=xmf, scalar=0.5, in1=fsl, op0=OP.mult, op1=OP.add
            )
            nc.sync.dma_start(
                out=outb.rearrange("(t p) c -> p t c", p=P)[:, i0 : i0 + cnt, :],
                in_=xmf,
            )

        for jt in range(NT):
            j0 = P * jt
            W = N - j0
            w_t = wpool.tile([P, N], f32, tag="w")
            nc.scalar.activation(
                out=w_t[:, :W], in_=a_src_b[:, j0:N], func=AF.Tanh,
                bias=bias_half[:, jt : jt + 1], scale=0.5,
            )
            ia_t = iapool.tile([P, N], f32, tag="ia")
            nc.vector.tensor_scalar(
                out=ia_t[:, :W], in0=w_t[:, :W], scalar1=-0.5, scalar2=-0.5,
                op0=OP.mult, op1=OP.add,
            )
            nc.sync.dma_start(out=IAT[j0 : j0 + P, j0:N], in_=ia_t[:, :W])

            e_t = singles.tile([P, W], bf16, tag=f"erow{jt}", name=f"erow{jt}")
            nc.scalar.activation(
                out=e_t, in_=w_t[:, :W], func=AF.Exp, scale=-0.5, bias=neg_half
            )
            m1_t = m1pool.tile([P, N], bf16, tag="m1")
            nc.sync.dma_start(out=m1_t[:, :W], in_=M1[j0 : j0 + P, j0:N])
            nc.vector.tensor_mul(out=e_t, in0=e_t, in1=m1_t[:, :W])
            nc.vector.tensor_add(out=e_t[:, 0:P], in0=e_t[:, 0:P], in1=m2_sb[:, jt, :])
            erows[jt] = e_t

        # tails: pairs for early tiles, singles for the last four (shorter
        # critical chains where they land on the kernel's tail)
        tail_groups = [(0, 2), (2, 2), (4, 2), (6, 2), (8, 2), (10, 2),
                       (12, 1), (13, 1), (14, 1), (15, 1)]
        it_to_group = {}
        for gi, (i0, cnt) in enumerate(tail_groups):
            for k in range(cnt):
                it_to_group[i0 + k] = gi

        # ---- P@h accumulation: dense matmul stream over retained rows ----
        for it in range(NT):
            acc = psum.tile([P, 512], f32, tag="acc", name=f"acc{it}")
            for jt in range(it + 1):
                off = P * (it - jt)
                nc.tensor.matmul(
                    acc[:, 0:DP3], lhsT=erows[jt][:, off : off + P],
                    rhs=h16[:, jt, :], start=(jt == 0), stop=False,
                )
            nc.tensor.matmul(
                acc[:, 0:DP3], lhsT=ones_row16, rhs=extra_row,
                start=False, stop=True,
            )
            g = it_to_group[it]
            i0, cnt = tail_groups[g]
            if it == i0:
                fpre_tiles[g] = fpool.tile(
                    [P, 2, D], f32, tag="fpre", name=f"fpre{g}",
                )[:, 0:cnt, :]
            rec = tpool.tile([P, 1], f32, tag="rec")
            nc.vector.reciprocal(out=rec, in_=acc[:, DP3 - 1 : DP3])
            nc.vector.scalar_tensor_tensor(
                out=fpre_tiles[g][:, it - i0, :], in0=acc[:, 0:D], scalar=rec,
                in1=bb_sb, op0=OP.mult, op1=OP.add,
            )
            if it == i0 + cnt - 1:
                emit_tail(g)

    nc.finalize()
    return nc


def _get_program():
    if "nc" not in _PROGRAM_CACHE:
        _PROGRAM_CACHE["nc"] = _build_program()
    return _PROGRAM_CACHE["nc"]


def _host_prep(feat_in, adj, W, b, Ws, bs, Hw, Hb):
    """Build the per-core input maps (layout marshalling + param prefolds)."""
    f4 = np.float32
    W0 = np.asarray(W, f4)[0]  # [256, 256] (in, out)
    ws_src = np.asarray(Ws, f4)[0, :D]
    ws_dst = np.asarray(Ws, f4)[0, D:]
    u_src = W0 @ ws_src
    u_dst = W0 @ ws_dst
    wbig = np.concatenate(
        [W0, u_src[:, None], u_dst[:, None], np.zeros((D, 1), f4)], axis=1
    ).astype(f4)
    crow = np.zeros((1, D + 3), f4)
    crow[0, D + 2] = 1.0  # ones column for h_aug (bs handled separately)
    hwt = np.ascontiguousarray(np.asarray(Hw, f4).T)
    hbr = np.asarray(Hb, f4).reshape(1, D)
    bb = np.ascontiguousarray(np.broadcast_to(np.asarray(b, f4), (P, D)))

    import ml_dtypes

    bf16 = ml_dtypes.bfloat16
    strict = np.triu(np.ones((N, N), dtype=f4), 1)  # [j, i] : j < i
    in_maps = []
    for bi in range(B):
        fb = np.ascontiguousarray(np.asarray(feat_in[bi], f4))
        fbT = np.ascontiguousarray(fb.T)
        adjb = np.asarray(adj[bi])
        # 0/1 keep-mask (exact in bf16); exp(-0.5) lives in the ACT exp bias
        m1 = np.ascontiguousarray((adjb.T.astype(f4)) * strict).astype(bf16)
        dvals = (np.diagonal(adjb).astype(f4)) * f4(E_DIAG)
        m2 = np.zeros((N, P), f4)
        m2[np.arange(N), np.arange(N) % P] = dvals
        m2 = m2.astype(bf16)
        in_maps.append(
            {
                "featT": fbT,
                "feat": fb,
                "m1": m1,
                "m2": m2,
                "wbig": wbig,
                "hwt": hwt.astype(bf16),
                "hbr": hbr.astype(bf16),
                "bb": bb,
                "crow": np.ascontiguousarray(np.broadcast_to(crow, (P, D + 3))),
                "bscol": np.full((P, 1), np.asarray(bs, f4)[0], f4),
            }
        )
    return in_maps


def _assemble(results):
    """Gather per-core outputs into full tensors (layout-only host work)."""
    f4 = np.float32
    out = np.stack([results[i]["outb"] for i in range(B)], axis=0)
    iat = np.stack([results[i]["iat"] for i in range(B)], axis=0)  # [B, j, i]
    ia_ij = iat.transpose(0, 2, 1)  # [B, i, j] view
    strict_low = np.tril(np.ones((N, N), dtype=bool), -1)
    eye = np.eye(N, dtype=f4)
    i_a_raw = np.where(strict_low[None], ia_ij, eye[None]).astype(f4)[:, None]
    return out.astype(f4), i_a_raw


def run_on_cores(in_maps, trace=False, **kwargs):
    from concourse.bass_utils import run_bass_kernel_spmd

    nc = _get_program()
    return run_bass_kernel_spmd(nc, in_maps, list(range(NCORES)), trace=trace, **kwargs)


def kernel(feat_in, adj, W, b, Ws, bs, Hw, Hb):
    in_maps = _host_prep(feat_in, adj, W, b, Ws, bs, Hw, Hb)
    res = run_on_cores(in_maps, trace=False)
    return _assemble(res.results)
